# revision 21
# baseline (speedup 1.0000x reference)
"""CrossViewRegionAligner Trainium2 kernel (fp16 device pipeline + exact host repair).

Device (8 NeuronCores, sat-axis sharded, 192 rows/core) computes the
[192, 1536] pairwise-MLP logit block per core in fp16:
  h1    = relu(hu_j + c_i)   fp16, DVE tensor_scalar 4x mode, 2 sat rows/instr
          (hu = uav @ W1[64:] replicated 2x across partitions; c = sat @ W1[:64]
           + b1 packed 2 rows/column -- both tiny [N,64] projections, 0.2% of
           the FLOPs, prepacked on host into the input blobs)
  h2    = relu(blockdiag(W2,W2)^T h1 + b2)       (PE fp16 1cyc/col -> PSUM fp32,
                                                  ACT relu+bias -> fp16 SBUF)
  logit = blockdiag-shifted W3 variants, 8-pass PSUM accumulation (PE fp16)

fp16 rounds logits by ~2e-3 worst-case (validated vs fp64 on the actual
inputs), far below most greedy top-2 gaps (median 4e-2) but above the
minimum (6.3e-5). Host greedy therefore recomputes, per row, every
candidate within 2*EPS of the approximate max in fp64 (numpy) and picks
the exact argmax -- identical picks to the fp32 reference as long as the
device error stays below EPS. ~2 candidates/row in practice.
"""

import os

import numpy as np

# If the axon NTFF profile hook is unavailable, a BASS_TRACE=1 environment
# would crash run_bass_kernel_spmd with ModuleNotFoundError -- disable
# tracing only in that case.
try:
    from antenv import axon_hooks as _axon_hooks  # noqa: F401
except Exception:
    os.environ.setdefault("BASS_NEVER_TRACE", "1")

import concourse.bass as bass
import concourse.bacc as bacc
import concourse.mybir as mybir
from concourse.tile import TileContext, add_dep_helper
from concourse.bass_utils import run_bass_kernel_spmd

FP = mybir.dt.float32
FP16 = mybir.dt.float16
N_SAT = 1536
N_UAV = 1536
D = 64
N_CORES = 8
RPC = N_SAT // N_CORES          # sat rows per core = 192
PASSES = RPC // 4               # 4 sat rows per pass = 48
GROUP = 8                       # passes per logit-accumulation group (32 rows)
N_GROUPS = PASSES // GROUP      # 6
CHUNK = 512                     # matmul free-dim chunk (one PSUM bank, fp32)
N_CHUNKS = N_UAV // CHUNK       # 3

# EPS: bound on |device logit - fp64 logit|. Measured max ~1.9e-3 on the
# actual inputs; 2.5e-2 leaves a 13x safety margin. Greedy repair cost
# grows only mildly with EPS (~2 candidates/row at 5e-2 net threshold).
EPS = 2.5e-2

# fp32 blob: c_pack [128, 96] + b2 packed [128, 1]
OFF_CP = 0
OFF_B2P = RPC // 2                  # 96
BLOB_W = OFF_B2P + 1                # 97

# fp16 blob: W2 blockdiag, huav replicated [128, 1536], shifted W3 variants
OFF16_W2P = 0
OFF16_HU = OFF16_W2P + 64           # 64
OFF16_W3P = OFF16_HU + N_UAV        # 1600
BLOB16_W = OFF16_W3P + GROUP * 32   # 1856

_CACHED_NC = None
LAST_RESULT = None  # BassKernelResults of the most recent run (for profiling)
LAST_SIM = None     # device logits of the most recent run (for eps checks)


def _build_nc():
    nc = bacc.Bacc(trn_type="TRN2")

    blob = nc.dram_tensor("blob", [128, BLOB_W], FP, kind="ExternalInput")
    blob16 = nc.dram_tensor("blob16", [128, BLOB16_W], FP16, kind="ExternalInput")
    louts = [
        nc.dram_tensor(f"lout{g}", [GROUP * 4, N_CHUNKS * CHUNK], FP,
                       kind="ExternalOutput")
        for g in range(N_GROUPS)
    ]

    with TileContext(nc) as tc:
        _body(nc, tc, blob, blob16, louts)
    nc.finalize()
    return nc


def _body(nc, tc, blob, blob16, louts):
    from contextlib import ExitStack

    H1S = 6    # h1 ring slots (per a/b buffer)
    H2S = 10   # h2 ring slots

    with ExitStack() as ctx:
        consts = ctx.enter_context(tc.tile_pool(name="consts", bufs=1))
        psum = ctx.enter_context(tc.tile_pool(name="psum", bufs=1, space="PSUM"))

        # ---- load constants, most-urgent first: w2p + huav chunk 0 (pass 0
        # starts on them), the tiny fp32 blob, the rest of huav, then w3p
        # (first needed at pass ~8). Separate DMAs => separate semaphores,
        # so pass 0 starts before the full huav transfer lands. -------------
        blob_sb = consts.tile([128, BLOB_W], FP, name="blob_sb", tag="blob")
        blob16_sb = consts.tile([128, BLOB16_W], FP16, name="blob16_sb", tag="blob16")
        cut = OFF16_HU + CHUNK
        nc.sync.dma_start(blob16_sb[:, 0:cut], blob16[:, 0:cut])
        nc.sync.dma_start(blob_sb[:], blob[:])
        nc.sync.dma_start(blob16_sb[:, cut:OFF16_W3P], blob16[:, cut:OFF16_W3P])
        nc.sync.dma_start(blob16_sb[:, OFF16_W3P:], blob16[:, OFF16_W3P:])

        c_pack = blob_sb[0:128, OFF_CP : OFF_CP + RPC // 2]
        b2p_sb = blob_sb[0:128, OFF_B2P : OFF_B2P + 1]
        huav_sb = blob16_sb[0:128, OFF16_HU : OFF16_HU + N_UAV]
        w2p_sb = blob16_sb[0:128, OFF16_W2P : OFF16_W2P + 64]
        w3p_sb = blob16_sb[0:128, OFF16_W3P : OFF16_W3P + GROUP * 32]

        # Permanent PSUM tiles (no pool recycling => no slot-transition
        # multi-waits; same-engine WAW is program order).
        psA = psum.tile([128, N_UAV], FP, name="psA", tag="psA")
        psB = psum.tile([128, N_UAV], FP, name="psB", tag="psB")
        lpA = psum.tile([GROUP * 4, CHUNK], FP, name="lpA", tag="lpA")
        lpB = psum.tile([GROUP * 4, CHUNK], FP, name="lpB", tag="lpB")

        # ACT consumes the fp32-blob DMA sem once (b2 bias source); DVE
        # consumes it via a tiny probe copy so the hot TS instructions only
        # ever wait on single semaphores.
        b2probe = consts.tile([128, 1], FP, name="b2probe", tag="b2probe")
        nc.scalar.copy(b2probe[:], b2p_sb[:])
        cprobe = consts.tile([128, 1], FP, name="cprobe", tag="cprobe")
        nc.vector.tensor_copy(cprobe[:], c_pack[:, 0:1])

        # Permanent SBUF rings (fp16)
        h1A = consts.tile([128, H1S * N_UAV], FP16, name="h1A", tag="h1A")
        h1B = consts.tile([128, H1S * N_UAV], FP16, name="h1B", tag="h1B")
        h2buf = consts.tile([128, H2S * N_UAV], FP16, name="h2buf", tag="h2buf")
        lsb = consts.tile([GROUP * 4, 4 * CHUNK], FP, name="lsb", tag="lsb")

        w2c = w2p_sb[:, 0:1]
        prev = {}  # last emitted instruction per engine, for chain edges

        def chain(key, binst):
            if key in prev:
                add_dep_helper(binst.ins, prev[key].ins, sync=False, reason="chain")
            prev[key] = binst
            return binst

        def emit_pass(t):
            """h1 (DVE), L2 matmuls (PE), relu drain (ACT) for pass t."""
            h1o = (t % H1S) * N_UAV
            h2o = (t % H2S) * N_UAV
            # pass 0 computes h1 per 512-chunk so the first L2 matmul can
            # start as soon as the first third of the huav DMA lands
            cks = [(c * CHUNK, CHUNK) for c in range(N_CHUNKS)] if t == 0 \
                else [(0, N_UAV)]
            for off, w in cks:
                chain("v", nc.vector.tensor_scalar(
                    out=h1A[:, h1o + off : h1o + off + w],
                    in0=huav_sb[:, off : off + w],
                    scalar1=c_pack[:, 2 * t : 2 * t + 1],
                    scalar2=0.0,
                    op0=mybir.AluOpType.add,
                    op1=mybir.AluOpType.max,
                ))
                chain("v", nc.vector.tensor_scalar(
                    out=h1B[:, h1o + off : h1o + off + w],
                    in0=huav_sb[:, off : off + w],
                    scalar1=c_pack[:, 2 * t + 1 : 2 * t + 2],
                    scalar2=0.0,
                    op0=mybir.AluOpType.add,
                    op1=mybir.AluOpType.max,
                ))
            ps = psA if t % 2 == 0 else psB
            chain("p", nc.tensor.matmul(
                ps[0:1, 0:1], w2c, w2c, skip_group_check=True
            ))
            for c in range(N_CHUNKS):
                sl = slice(c * CHUNK, (c + 1) * CHUNK)
                hslc = slice(h1o + c * CHUNK, h1o + (c + 1) * CHUNK)
                chain("p", nc.tensor.matmul(ps[0:64, sl], w2p_sb[:], h1A[:, hslc]))
                chain("p", nc.tensor.matmul(ps[64:128, sl], w2p_sb[:], h1B[:, hslc]))
            chain("a", nc.scalar.activation(
                h2buf[:, h2o : h2o + N_UAV],
                ps[:],
                mybir.ActivationFunctionType.Relu,
                bias=b2p_sb[:],
            ))

        def emit_l3_chunk(g, c):
            """Accumulate group g / chunk c into PSUM, stage, DMA out."""
            n = g * N_CHUNKS + c
            lp = lpA if n % 2 == 0 else lpB
            for q in range(GROUP):
                p = g * GROUP + q
                ho = (p % H2S) * N_UAV + c * CHUNK
                chain("p", nc.tensor.matmul(
                    lp[:],
                    w3p_sb[:, q * 32 : (q + 1) * 32],
                    h2buf[:, ho : ho + CHUNK],
                    start=(q == 0),
                    stop=(q == GROUP - 1),
                ))
            # stage in SBUF (4-slot ring), then overlapped per-chunk DMA
            sl = lsb[:, (n % 4) * CHUNK : (n % 4 + 1) * CHUNK]
            chain("a", nc.scalar.copy(sl, lp[:]))
            nc.sync.dma_start(louts[g][:, c * CHUNK : (c + 1) * CHUNK], sl)

        # Main loop. Each group's L3 chunks are lagged into the next group's
        # first two passes so PE always has L2 work queued while ACT drains,
        # and the per-chunk DMA semaphores have a pass of slack to land.
        # Lag is capped at 2 passes: pass g*8+10 reuses h2buf slot g*8 % H2S.
        for t in range(PASSES):
            emit_pass(t)
            g = t // GROUP - 1
            if g >= 0 and t % GROUP == 0:
                emit_l3_chunk(g, 0)
            elif g >= 0 and t % GROUP == 1:
                emit_l3_chunk(g, 1)
                emit_l3_chunk(g, 2)
        for c in range(N_CHUNKS):
            emit_l3_chunk(N_GROUPS - 1, c)


def _prepack(sat_shard, uav_regions, W1, b1, W2, b2, W3):
    f32 = np.float32
    f16 = np.float16
    W1a, W1b = W1[:D], W1[D:]

    # c_pack: col t = (c_{4t?}...) -- col t packs rows (2t, 2t+1):
    # partitions 0-63 = c[2t], 64-127 = c[2t+1]
    c = (sat_shard.astype(f32) @ W1a.astype(f32) + b1.astype(f32))  # [192, 64]
    blob = np.zeros((128, BLOB_W), f32)
    blob[0:D, OFF_CP : OFF_CP + RPC // 2] = c[0::2].T
    blob[D:128, OFF_CP : OFF_CP + RPC // 2] = c[1::2].T
    blob[:, OFF_B2P] = np.tile(b2, 4)

    hu = uav_regions.astype(f32) @ W1b.astype(f32)                  # [1536, 64]
    blob16 = np.zeros((128, BLOB16_W), f16)
    blob16[0:D, OFF16_HU : OFF16_HU + N_UAV] = hu.T
    blob16[D:128, OFF16_HU : OFF16_HU + N_UAV] = hu.T
    blob16[0:D, OFF16_W2P : OFF16_W2P + 32] = W2
    blob16[D:128, OFF16_W2P + 32 : OFF16_W2P + 64] = W2
    # variant q places blockdiag(W3 x4) rows at output columns 4q..4q+3
    for q in range(GROUP):
        for r in range(4):
            blob16[32 * r : 32 * (r + 1), OFF16_W3P + q * 32 + 4 * q + r] = W3[:, 0]
    return dict(blob=np.ascontiguousarray(blob),
                blob16=np.ascontiguousarray(blob16))


def _exact_logits(sat_rows, uav_rows, W1, b1, W2, b2, W3, b3):
    """fp64 logits for given sat rows x uav rows (repair path)."""
    d = W1.shape[0] // 2
    c = sat_rows @ W1[:d] + b1            # [n, 64]
    hu = uav_rows @ W1[d:]                # [m, 64]
    h1 = np.maximum(c[:, None, :] + hu[None, :, :], 0.0)
    h2 = np.maximum(h1 @ W2 + b2, 0.0)
    return h2 @ W3[:, 0] + b3[0]          # [n, m]


def _greedy_assign_repair(sim, sat, uav, W1, b1, W2, b2, W3, b3):
    """Sequential greedy matching with exact fp64 repair of near-ties.

    sim: approximate logits with |sim - logit_f64| <= EPS elementwise
    (up to a uniform per-matrix shift, which cancels in comparisons).
    Any candidate within 2*EPS of a row's running max may be the true
    argmax; recompute those exactly.
    """
    f64 = np.float64
    satd, uavd = sat.astype(f64), uav.astype(f64)
    W1d, b1d = W1.astype(f64), b1.astype(f64)
    W2d, b2d = W2.astype(f64), b2.astype(f64)
    W3d, b3d = W3.astype(f64), b3.astype(f64)

    scores = sim.astype(np.float32).copy()
    n, m = scores.shape
    assign = np.empty(n, np.int64)
    n_repair = 0
    for i in range(n):
        row = scores[i]
        jmax = int(np.argmax(row))
        top = row[jmax]
        cand = np.nonzero(row >= top - 2.0 * EPS)[0]
        if len(cand) > 1:
            n_repair += len(cand)
            exact = _exact_logits(satd[i : i + 1], uavd[cand],
                                  W1d, b1d, W2d, b2d, W3d, b3d)[0]
            jmax = int(cand[np.argmax(exact)])
        assign[i] = jmax
        scores[:, jmax] = -np.inf
    _greedy_assign_repair.last_n_repair = n_repair
    return assign


def kernel(sat_regions, uav_regions, W1, b1, W2, b2, W3, b3):
    global _CACHED_NC
    if _CACHED_NC is None:
        _CACHED_NC = _build_nc()
    nc = _CACHED_NC

    in_maps = []
    for k in range(N_CORES):
        shard = sat_regions[k * RPC : (k + 1) * RPC]
        in_maps.append(_prepack(shard, uav_regions, W1, b1, W2, b2, W3))

    res = run_bass_kernel_spmd(nc, in_maps, core_ids=list(range(N_CORES)))
    global LAST_RESULT
    LAST_RESULT = res
    sim = np.empty((N_SAT, N_UAV), np.float32)
    for k in range(N_CORES):
        for g in range(N_GROUPS):
            la = res.results[k][f"lout{g}"]  # [32, 3*512]
            sim[k * RPC + g * GROUP * 4 : k * RPC + (g + 1) * GROUP * 4] = la

    global LAST_SIM
    LAST_SIM = sim
    assign = _greedy_assign_repair(
        sim, sat_regions, uav_regions, W1, b1, W2, b2, W3, b3)
    out = np.stack([sat_regions, uav_regions[assign]], axis=1)
    return np.ascontiguousarray(out, dtype=np.float32)


# revision 22
# speedup vs baseline: 1.0148x; 1.0148x over previous
"""CrossViewRegionAligner Trainium2 kernel (fp16 device pipeline + exact host repair).

Device (8 NeuronCores, sat-axis sharded, 192 rows/core) computes the
[192, 1536] pairwise-MLP logit block per core in fp16:
  h1    = relu(hu_j + c_i)   fp16, DVE tensor_scalar 4x mode, 2 sat rows/instr
          (hu = uav @ W1[64:] replicated 2x across partitions; c = sat @ W1[:64]
           + b1 packed 2 rows/column -- both tiny [N,64] projections, 0.2% of
           the FLOPs, prepacked on host into the input blobs)
  h2    = relu(blockdiag(W2,W2)^T h1 + b2)       (PE fp16 1cyc/col -> PSUM fp32,
                                                  ACT relu+bias -> fp16 SBUF)
  logit = blockdiag-shifted W3 variants, 8-pass PSUM accumulation (PE fp16)

fp16 rounds logits by ~2e-3 worst-case (validated vs fp64 on the actual
inputs), far below most greedy top-2 gaps (median 4e-2) but above the
minimum (6.3e-5). Host greedy therefore recomputes, per row, every
candidate within 2*EPS of the approximate max in fp64 (numpy) and picks
the exact argmax -- identical picks to the fp32 reference as long as the
device error stays below EPS. ~2 candidates/row in practice.
"""

import os

import numpy as np

# If the axon NTFF profile hook is unavailable, a BASS_TRACE=1 environment
# would crash run_bass_kernel_spmd with ModuleNotFoundError -- disable
# tracing only in that case.
try:
    from antenv import axon_hooks as _axon_hooks  # noqa: F401
except Exception:
    os.environ.setdefault("BASS_NEVER_TRACE", "1")

import concourse.bass as bass
import concourse.bacc as bacc
import concourse.mybir as mybir
from concourse.tile import TileContext, add_dep_helper
from concourse.bass_utils import run_bass_kernel_spmd

FP = mybir.dt.float32
FP16 = mybir.dt.float16
N_SAT = 1536
N_UAV = 1536
D = 64
N_CORES = 8
RPC = N_SAT // N_CORES          # sat rows per core = 192
PASSES = RPC // 4               # 4 sat rows per pass = 48
GROUP = 8                       # passes per logit-accumulation group (32 rows)
N_GROUPS = PASSES // GROUP      # 6
CHUNK = 512                     # matmul free-dim chunk (one PSUM bank, fp32)
N_CHUNKS = N_UAV // CHUNK       # 3

# EPS: bound on |device logit - fp64 logit|. Measured max ~1.9e-3 on the
# actual inputs; 2.5e-2 leaves a 13x safety margin. Greedy repair cost
# grows only mildly with EPS (~2 candidates/row at 5e-2 net threshold).
EPS = 2.5e-2

# fp32 blob: c_pack [128, 96] + b2 packed [128, 1]
OFF_CP = 0
OFF_B2P = RPC // 2                  # 96
BLOB_W = OFF_B2P + 1                # 97

# fp16 blob: W2 blockdiag, huav replicated [128, 1536], shifted W3 variants
OFF16_W2P = 0
OFF16_HU = OFF16_W2P + 64           # 64
OFF16_W3P = OFF16_HU + N_UAV        # 1600
BLOB16_W = OFF16_W3P + GROUP * 32   # 1856

_CACHED_NC = None
LAST_RESULT = None  # BassKernelResults of the most recent run (for profiling)
LAST_SIM = None     # device logits of the most recent run (for eps checks)


def _build_nc():
    nc = bacc.Bacc(trn_type="TRN2")

    blob = nc.dram_tensor("blob", [128, BLOB_W], FP, kind="ExternalInput")
    blob16 = nc.dram_tensor("blob16", [128, BLOB16_W], FP16, kind="ExternalInput")
    louts = [
        nc.dram_tensor(f"lout{g}", [GROUP * 4, N_CHUNKS * CHUNK], FP,
                       kind="ExternalOutput")
        for g in range(N_GROUPS)
    ]

    with TileContext(nc) as tc:
        _body(nc, tc, blob, blob16, louts)
    nc.finalize()
    return nc


def _body(nc, tc, blob, blob16, louts):
    from contextlib import ExitStack

    H1S = 6    # h1 ring slots (per a/b buffer)
    H2S = 12   # h2 ring slots

    with ExitStack() as ctx:
        consts = ctx.enter_context(tc.tile_pool(name="consts", bufs=1))
        psum = ctx.enter_context(tc.tile_pool(name="psum", bufs=1, space="PSUM"))

        # ---- load constants, most-urgent first: w2p + huav chunk 0 (pass 0
        # starts on them), the tiny fp32 blob, the rest of huav, then w3p
        # (first needed at pass ~8). Separate DMAs => separate semaphores,
        # so pass 0 starts before the full huav transfer lands. -------------
        blob_sb = consts.tile([128, BLOB_W], FP, name="blob_sb", tag="blob")
        blob16_sb = consts.tile([128, BLOB16_W], FP16, name="blob16_sb", tag="blob16")
        cut = OFF16_HU + CHUNK
        nc.sync.dma_start(blob16_sb[:, 0:cut], blob16[:, 0:cut])
        nc.sync.dma_start(blob_sb[:], blob[:])
        nc.sync.dma_start(blob16_sb[:, cut:OFF16_W3P], blob16[:, cut:OFF16_W3P])
        nc.sync.dma_start(blob16_sb[:, OFF16_W3P:], blob16[:, OFF16_W3P:])

        c_pack = blob_sb[0:128, OFF_CP : OFF_CP + RPC // 2]
        b2p_sb = blob_sb[0:128, OFF_B2P : OFF_B2P + 1]
        huav_sb = blob16_sb[0:128, OFF16_HU : OFF16_HU + N_UAV]
        w2p_sb = blob16_sb[0:128, OFF16_W2P : OFF16_W2P + 64]
        w3p_sb = blob16_sb[0:128, OFF16_W3P : OFF16_W3P + GROUP * 32]

        # Permanent PSUM tiles (no pool recycling => no slot-transition
        # multi-waits; same-engine WAW is program order).
        psA = psum.tile([128, N_UAV], FP, name="psA", tag="psA")
        psB = psum.tile([128, N_UAV], FP, name="psB", tag="psB")
        lpA = psum.tile([GROUP * 4, CHUNK], FP, name="lpA", tag="lpA")
        lpB = psum.tile([GROUP * 4, CHUNK], FP, name="lpB", tag="lpB")

        # ACT consumes the fp32-blob DMA sem once (b2 bias source); DVE
        # consumes it via a tiny probe copy so the hot TS instructions only
        # ever wait on single semaphores.
        b2probe = consts.tile([128, 1], FP, name="b2probe", tag="b2probe")
        nc.scalar.copy(b2probe[:], b2p_sb[:])
        cprobe = consts.tile([128, 1], FP, name="cprobe", tag="cprobe")
        nc.vector.tensor_copy(cprobe[:], c_pack[:, 0:1])

        # Permanent SBUF rings (fp16)
        h1A = consts.tile([128, H1S * N_UAV], FP16, name="h1A", tag="h1A")
        h1B = consts.tile([128, H1S * N_UAV], FP16, name="h1B", tag="h1B")
        h2buf = consts.tile([128, H2S * N_UAV], FP16, name="h2buf", tag="h2buf")
        lsb = consts.tile([GROUP * 4, 4 * CHUNK], FP, name="lsb", tag="lsb")

        w2c = w2p_sb[:, 0:1]
        prev = {}  # last emitted instruction per engine, for chain edges

        def chain(key, binst):
            if key in prev:
                add_dep_helper(binst.ins, prev[key].ins, sync=False, reason="chain")
            prev[key] = binst
            return binst

        def emit_pass(t):
            """h1 (DVE), L2 matmuls (PE), relu drain (ACT) for pass t."""
            h1o = (t % H1S) * N_UAV
            h2o = (t % H2S) * N_UAV
            # pass 0 computes h1 per 512-chunk so the first L2 matmul can
            # start as soon as the first third of the huav DMA lands
            cks = [(c * CHUNK, CHUNK) for c in range(N_CHUNKS)] if t == 0 \
                else [(0, N_UAV)]
            for off, w in cks:
                chain("v", nc.vector.tensor_scalar(
                    out=h1A[:, h1o + off : h1o + off + w],
                    in0=huav_sb[:, off : off + w],
                    scalar1=c_pack[:, 2 * t : 2 * t + 1],
                    scalar2=0.0,
                    op0=mybir.AluOpType.add,
                    op1=mybir.AluOpType.max,
                ))
                chain("v", nc.vector.tensor_scalar(
                    out=h1B[:, h1o + off : h1o + off + w],
                    in0=huav_sb[:, off : off + w],
                    scalar1=c_pack[:, 2 * t + 1 : 2 * t + 2],
                    scalar2=0.0,
                    op0=mybir.AluOpType.add,
                    op1=mybir.AluOpType.max,
                ))
            ps = psA if t % 2 == 0 else psB
            chain("p", nc.tensor.matmul(
                ps[0:1, 0:1], w2c, w2c, skip_group_check=True
            ))
            for c in range(N_CHUNKS):
                sl = slice(c * CHUNK, (c + 1) * CHUNK)
                hslc = slice(h1o + c * CHUNK, h1o + (c + 1) * CHUNK)
                chain("p", nc.tensor.matmul(ps[0:64, sl], w2p_sb[:], h1A[:, hslc]))
                chain("p", nc.tensor.matmul(ps[64:128, sl], w2p_sb[:], h1B[:, hslc]))
            chain("a", nc.scalar.activation(
                h2buf[:, h2o : h2o + N_UAV],
                ps[:],
                mybir.ActivationFunctionType.Relu,
                bias=b2p_sb[:],
            ))

        def emit_l3_chunk(g, c):
            """Accumulate group g / chunk c into PSUM, stage, DMA out."""
            n = g * N_CHUNKS + c
            lp = lpA if n % 2 == 0 else lpB
            for q in range(GROUP):
                p = g * GROUP + q
                ho = (p % H2S) * N_UAV + c * CHUNK
                chain("p", nc.tensor.matmul(
                    lp[:],
                    w3p_sb[:, q * 32 : (q + 1) * 32],
                    h2buf[:, ho : ho + CHUNK],
                    start=(q == 0),
                    stop=(q == GROUP - 1),
                ))
            # stage in SBUF (4-slot ring), then overlapped per-chunk DMA
            sl = lsb[:, (n % 4) * CHUNK : (n % 4 + 1) * CHUNK]
            chain("a", nc.scalar.copy(sl, lp[:]))
            nc.sync.dma_start(louts[g][:, c * CHUNK : (c + 1) * CHUNK], sl)

        # Main loop. Each group's L3 chunks are lagged into the next group's
        # first three passes so PE always has L2 work queued while ACT
        # drains, and the per-chunk DMA semaphores have a pass of slack to
        # land. Lag is capped at H2S-8 passes (h2buf slot reuse).
        for t in range(PASSES):
            emit_pass(t)
            g = t // GROUP - 1
            if g >= 0 and t % GROUP <= 2:
                emit_l3_chunk(g, t % GROUP)
        for c in range(N_CHUNKS):
            emit_l3_chunk(N_GROUPS - 1, c)


def _prepack(sat_shard, uav_regions, W1, b1, W2, b2, W3):
    f32 = np.float32
    f16 = np.float16
    W1a, W1b = W1[:D], W1[D:]

    # c_pack: col t = (c_{4t?}...) -- col t packs rows (2t, 2t+1):
    # partitions 0-63 = c[2t], 64-127 = c[2t+1]
    c = (sat_shard.astype(f32) @ W1a.astype(f32) + b1.astype(f32))  # [192, 64]
    blob = np.zeros((128, BLOB_W), f32)
    blob[0:D, OFF_CP : OFF_CP + RPC // 2] = c[0::2].T
    blob[D:128, OFF_CP : OFF_CP + RPC // 2] = c[1::2].T
    blob[:, OFF_B2P] = np.tile(b2, 4)

    hu = uav_regions.astype(f32) @ W1b.astype(f32)                  # [1536, 64]
    blob16 = np.zeros((128, BLOB16_W), f16)
    blob16[0:D, OFF16_HU : OFF16_HU + N_UAV] = hu.T
    blob16[D:128, OFF16_HU : OFF16_HU + N_UAV] = hu.T
    blob16[0:D, OFF16_W2P : OFF16_W2P + 32] = W2
    blob16[D:128, OFF16_W2P + 32 : OFF16_W2P + 64] = W2
    # variant q places blockdiag(W3 x4) rows at output columns 4q..4q+3
    for q in range(GROUP):
        for r in range(4):
            blob16[32 * r : 32 * (r + 1), OFF16_W3P + q * 32 + 4 * q + r] = W3[:, 0]
    return dict(blob=np.ascontiguousarray(blob),
                blob16=np.ascontiguousarray(blob16))


def _exact_logits(sat_rows, uav_rows, W1, b1, W2, b2, W3, b3):
    """fp64 logits for given sat rows x uav rows (repair path)."""
    d = W1.shape[0] // 2
    c = sat_rows @ W1[:d] + b1            # [n, 64]
    hu = uav_rows @ W1[d:]                # [m, 64]
    h1 = np.maximum(c[:, None, :] + hu[None, :, :], 0.0)
    h2 = np.maximum(h1 @ W2 + b2, 0.0)
    return h2 @ W3[:, 0] + b3[0]          # [n, m]


def _greedy_assign_repair(sim, sat, uav, W1, b1, W2, b2, W3, b3):
    """Sequential greedy matching with exact fp64 repair of near-ties.

    sim: approximate logits with |sim - logit_f64| <= EPS elementwise
    (up to a uniform per-matrix shift, which cancels in comparisons).
    Any candidate within 2*EPS of a row's running max may be the true
    argmax; recompute those exactly.
    """
    f64 = np.float64
    satd, uavd = sat.astype(f64), uav.astype(f64)
    W1d, b1d = W1.astype(f64), b1.astype(f64)
    W2d, b2d = W2.astype(f64), b2.astype(f64)
    W3d, b3d = W3.astype(f64), b3.astype(f64)

    scores = sim.astype(np.float32).copy()
    n, m = scores.shape
    assign = np.empty(n, np.int64)
    n_repair = 0
    for i in range(n):
        row = scores[i]
        jmax = int(np.argmax(row))
        top = row[jmax]
        cand = np.nonzero(row >= top - 2.0 * EPS)[0]
        if len(cand) > 1:
            n_repair += len(cand)
            exact = _exact_logits(satd[i : i + 1], uavd[cand],
                                  W1d, b1d, W2d, b2d, W3d, b3d)[0]
            jmax = int(cand[np.argmax(exact)])
        assign[i] = jmax
        scores[:, jmax] = -np.inf
    _greedy_assign_repair.last_n_repair = n_repair
    return assign


def kernel(sat_regions, uav_regions, W1, b1, W2, b2, W3, b3):
    global _CACHED_NC
    if _CACHED_NC is None:
        _CACHED_NC = _build_nc()
    nc = _CACHED_NC

    in_maps = []
    for k in range(N_CORES):
        shard = sat_regions[k * RPC : (k + 1) * RPC]
        in_maps.append(_prepack(shard, uav_regions, W1, b1, W2, b2, W3))

    res = run_bass_kernel_spmd(nc, in_maps, core_ids=list(range(N_CORES)))
    global LAST_RESULT
    LAST_RESULT = res
    sim = np.empty((N_SAT, N_UAV), np.float32)
    for k in range(N_CORES):
        for g in range(N_GROUPS):
            la = res.results[k][f"lout{g}"]  # [32, 3*512]
            sim[k * RPC + g * GROUP * 4 : k * RPC + (g + 1) * GROUP * 4] = la

    global LAST_SIM
    LAST_SIM = sim
    assign = _greedy_assign_repair(
        sim, sat_regions, uav_regions, W1, b1, W2, b2, W3, b3)
    out = np.stack([sat_regions, uav_regions[assign]], axis=1)
    return np.ascontiguousarray(out, dtype=np.float32)
